# revision 7
# baseline (speedup 1.0000x reference)
"""PoissonGaussianReadout forward on 8 trn2 NeuronCores.

Math (eval mode): each neuron n samples feat[b] (a [36,36,1024] image per
batch, 1024 = C*T channels) bilinearly at a fixed point mu[n], then takes a
per-neuron dot with W[n,:], adds b[n], applies elu(y)+1.

Strategy:
  - Hybrid shard 4x2: 8 cores = 4 batch-groups (4 batches each) x 2 halves
    of the contraction dim D (512 channels each).  Cores emit LINEAR
    partial sums; the host adds the halves, bias, and elu on [16,4096].
  - fp8(e4m3) x and W with DoubleRow matmuls; per-neuron dequant folded
    into a bf16 mask.
  - Sort neurons by bilinear base cell p00; 32 blocks of 128 neurons,
    each spanning a window of <=93 flat positions.  Two DoubleRow
    matmuls per block: psum[n, (b,j)] += Wblk^T @ feat-window.
  - The bilinear mask-reduce runs entirely on DVE as fused
    scalar_tensor_tensor+accum chains straight out of PSUM (~156ns per
    (block,batch)).  Measured on HW, every offload (GpSimd tensor_tensor
    mult ~775ns/blk, Act activation+accum ~380+279ns/batch, DVE
    tensor_reduce ~470ns/blk with op-mixing penalties) loses to this.
  - DMA: ONE need-ordered, ungated stream on the sync-engine HWDGE queue.
    Each block group ships as a single combined [W|F|M] host buffer,
    DMA'd as TWO big half-entries: a lone queue entry only sustains
    ~0.2 MB/us while two concurrent entries reach the ~0.4 MB/us per-core
    cap, and more than ~8 total entries stall on Tile's DMA-semaphore
    reuse.  Ragged groups (2,4,9,9,8 blocks) start the pipeline early.
    The two HWDGE queues share one bandwidth pool, so the scalar queue
    carries only the (late, tiny) z stores.
"""
import sys
sys.path.insert(0, "/opt/trn_rl_repo")

import numpy as np

from concourse import bass, mybir, tile
from concourse.bass_utils import run_bass_kernel_spmd
import bass_rust

# problem constants
B, C, T, HH, WW = 16, 64, 16, 36, 36
N, D = 4096, C * T             # 4096 neurons, 1024 input dim
P = HH * WW                    # 1296 flat positions
NCORES = 8
NBG = 4                        # batch groups
NDH = 2                        # D halves
BPC = B // NBG                 # batches per core = 4
DH = D // NDH                  # channels per core = 512
NC2 = DH // 256                # 2 double-subtile (256-chan) passes per core
PAD = 38                       # max corner offset (37) + 1
WINMAX = 128                   # psum bank: BPC*WIN <= 512 fp32
GSIZES = (2, 4, 9, 9, 8)       # ragged block groups: small first groups so
                               # the reduce pipeline starts earlier

F32 = mybir.dt.float32
BF16 = mybir.dt.bfloat16

import ml_dtypes
F8_DT = mybir.dt.float8e4
F8_NP = ml_dtypes.float8_e4m3   # max normal 240
F8_CAP = np.float32(224.0)


def _split_waits(nc, max_waits=1):
    """Walrus in this image allows only ONE sem wait per instruction.
    Hoist extra waits onto injected same-engine NoOps placed immediately
    before the owning instruction (same engine + program order => same
    semantics)."""
    k = 0
    for fn in nc.m.functions:
        for blk in fn.blocks:
            insts = blk.instructions
            out = []
            for inst in insts:
                si = inst.sync_info
                if si is not None and si.on_wait and len(si.on_wait) > max_waits:
                    waits = list(si.on_wait)
                    for w in waits[:-max_waits]:
                        nop = mybir.InstNoOp(name=f"I-wsplit-{k}", ins=[], outs=[])
                        k += 1
                        nop.engine = inst.engine
                        nop.sync_info = bass_rust.SyncInfo(
                            on_wait=[w], on_update=[]
                        )
                        out.append(nop)
                    si.on_wait = waits[-max_waits:]
                    inst.sync_info = si
                out.append(inst)
            if len(out) != len(insts):
                insts.clear()
                insts.extend(out)


def _bilinear_tables(mu):
    """Per-neuron base cell p00, corner offsets (4) in {0,1,36,37}, corner
    weights (4), replicating reference float32 arithmetic exactly."""
    one, half = np.float32(1.0), np.float32(0.5)
    g = np.clip(mu.astype(np.float32), -one, one)
    ix = (g[:, 0] + one) * np.float32(WW * 0.5) - half
    iy = (g[:, 1] + one) * np.float32(HH * 0.5) - half
    x0 = np.floor(ix)
    y0 = np.floor(iy)
    wx1 = ix - x0
    wx0 = one - wx1
    wy1 = iy - y0
    wy0 = one - wy1

    xs = [x0, x0 + one]
    ys = [y0, y0 + one]
    wxs = [wx0, wx1]
    wys = [wy0, wy1]

    x0c = np.clip(x0, 0, WW - 1).astype(np.int64)
    y0c = np.clip(y0, 0, HH - 1).astype(np.int64)
    p00 = y0c * WW + x0c

    offs = np.zeros((4, N), np.int64)
    wgts = np.zeros((4, N), np.float32)
    k = 0
    for a in range(2):          # y corner
        for bb in range(2):     # x corner
            xx, yy = xs[bb], ys[a]
            valid = (xx >= 0) & (xx <= WW - 1) & (yy >= 0) & (yy <= HH - 1)
            xi = np.clip(xx, 0, WW - 1).astype(np.int64)
            yi = np.clip(yy, 0, HH - 1).astype(np.int64)
            offs[k] = yi * WW + xi - p00
            wgts[k] = (wys[a] * wxs[bb]) * valid.astype(np.float32)
            k += 1
    assert offs.min() >= 0 and offs.max() <= 37
    return p00, offs, wgts


def _make_blocks(p00_sorted):
    """Greedy blocks of <=128 sorted neurons with window <= WINMAX."""
    blocks = []  # (start, end) into sorted order
    s = 0
    n = len(p00_sorted)
    while s < n:
        pfirst = p00_sorted[s]
        e = s
        while e < n and e - s < 128 and (p00_sorted[e] - pfirst) + PAD <= WINMAX:
            e += 1
        blocks.append((s, e))
        s = e
    return blocks


def kernel(x, mu, sigma, W, b):
    x = np.ascontiguousarray(x, dtype=np.float32)
    W = np.ascontiguousarray(W, dtype=np.float32)
    b = np.asarray(b, dtype=np.float32)

    p00, offs, wgts = _bilinear_tables(mu)
    order = np.argsort(p00, kind="stable")
    p00s = p00[order]
    blocks = _make_blocks(p00s)
    nblk = len(blocks)
    gbounds = [0]
    for gs in GSIZES:
        gbounds.append(min(gbounds[-1] + gs, nblk))
    while gbounds[-1] < nblk:
        gbounds.append(min(gbounds[-1] + GSIZES[-1], nblk))
    gbounds = sorted(set(gbounds))
    ngrp = len(gbounds) - 1

    # ---- fp8 quantization: global x scale, per-neuron W scale ----
    sx = F8_CAP / np.float32(max(np.abs(x).max(), 1e-30))
    sw = F8_CAP / np.maximum(np.abs(W).max(axis=1), 1e-30).astype(np.float32)
    Wq = (W * sw[:, None]).astype(F8_NP)    # [N, D]
    dequant = 1.0 / (sw * sx)               # [N] folded into the mask

    # per-block host data
    wins, pfirsts, ms, sblk = [], [], [], []
    mparts = []
    for i, (s, e) in enumerate(blocks):
        idx = order[s:e]
        m = e - s
        pfirst = int(p00s[s])
        win = int(p00s[e - 1]) - pfirst + PAD
        ms.append(m)
        pfirsts.append(pfirst)
        wins.append(win)
        sblk.append(s)
        # mask [128, win], fp8 dequant folded in
        mk = np.zeros((128, win), np.float32)
        rel = (p00[idx] - pfirst)  # [m]
        for k in range(4):
            np.add.at(mk[:m], (np.arange(m), rel + offs[k][idx]),
                      wgts[k][idx] * dequant[idx])
        mparts.append(mk)
    sblk.append(N)

    mask_all = np.ascontiguousarray(
        np.concatenate(mparts, axis=1)).astype(ml_dtypes.bfloat16)
    moffs = np.cumsum([0] + [w for w in wins])
    mtot = int(mask_all.shape[1])

    # feat segments: one per block group; window-union of its blocks
    seg_lo, seg_w = [], []
    for g in range(ngrp):
        lo = pfirsts[gbounds[g]]
        hi = max(pfirsts[i] + wins[i] for i in range(gbounds[g], gbounds[g + 1]))
        seg_lo.append(lo)
        seg_w.append(hi - lo)

    # W packed per group with one contiguous row per partition:
    # group layout [128, NC2, 2, sum_m(group)].
    Ws = Wq[order]                          # [N, D] sorted
    gms = [sblk[gbounds[g + 1]] - sblk[gbounds[g]] for g in range(ngrp)]
    wgrps = []                              # [dh][g] -> [128, NC2, 2, gm]
    for dh in range(NDH):
        wl = (Ws[:, dh * DH:(dh + 1) * DH].T        # [512, N]
              .reshape(NC2, 2, 128, N).transpose(2, 0, 1, 3))  # [128,NC2,2,N]
        parts = []
        for g in range(ngrp):
            lo, hi = sblk[gbounds[g]], sblk[gbounds[g + 1]]
            parts.append(np.ascontiguousarray(wl[:, :, :, lo:hi]))
        wgrps.append(parts)

    # per-group combined stream buffer: [ W | F | M ] bytes, one contiguous
    # row per partition, DMA'd as two big halves -> only 2 queue entries per
    # group (Tile has ~8 DMA sems; more entries serialize on sem reuse and
    # starve the queue).
    woffs, foffs, muoffs, gcols = [], [], [], []
    for g in range(ngrp):
        wb = NC2 * 2 * gms[g]
        fb = NC2 * 2 * BPC * seg_w[g]
        mb = 2 * int(moffs[gbounds[g + 1]] - moffs[gbounds[g]])
        woffs.append(0)
        foffs.append(wb)
        muoffs.append(wb + fb)
        gcols.append(wb + fb + mb)

    # ---- build the Bass program (same for all cores) ----
    nc = bass.Bass()
    gq_hs = [nc.declare_dram_parameter(f"gq{g}", [128, gcols[g]], F8_DT,
                                       isOutput=False)
             for g in range(ngrp)]
    z_h = nc.declare_dram_parameter("z", [128, BPC * nblk], F32, isOutput=True)

    ADD = mybir.AluOpType.add
    MULT = mybir.AluOpType.mult
    DR = mybir.MatmulPerfMode.DoubleRow

    with tile.TileContext(nc) as tc:
        with (
            tc.tile_pool(name="gq", bufs=1) as gqp,
            tc.tile_pool(name="spool", bufs=4) as spool,
            tc.tile_pool(name="zpool", bufs=1) as zpool,
            tc.tile_pool(name="psum", bufs=1, space="PSUM") as psump,
        ):
            gts = [gqp.tile([128, gcols[g]], F8_DT, name=f"gq{g}")
                   for g in range(ngrp)]
            gbt = [gt.bitcast(BF16) for gt in gts]   # bf16 view for masks
            zAll = zpool.tile([128, BPC * nblk], F32)

            # ONE need-ordered, ungated stream on the sync queue: two big
            # half-entries per group keep the queue 2-deep.
            for g in range(ngrp):
                h = (gcols[g] // 2) & ~1
                nc.sync.dma_start(gts[g][:, 0:h], gq_hs[g][:, 0:h])
                nc.sync.dma_start(gts[g][:, h:gcols[g]],
                                  gq_hs[g][:, h:gcols[g]])

            def w_ap(g, c, o, m):
                # [128, 2, m] fp8 view of the W part of group g, chunk c
                base = gts[g][:, 0:1]
                pstr = list(base.ap)[0]
                gm = gms[g]
                return bass.AP(base.tensor,
                               base.offset + woffs[g] + c * 2 * gm + o,
                               [list(pstr), [gm, 2], [1, m]])

            def f_ap(g, c, off, win):
                # [128, 2, BPC, win] fp8 view of the feat part of group g
                base = gts[g][:, 0:1]
                pstr = list(base.ap)[0]
                sw_ = seg_w[g]
                return bass.AP(base.tensor,
                               base.offset + foffs[g]
                               + c * 2 * BPC * sw_ + off,
                               [list(pstr), [BPC * sw_, 2], [sw_, BPC],
                                [1, win]])

            def m_ap(g, m, lo, win):
                # [m, win] bf16 view of the mask part of group g
                e0 = muoffs[g] // 2 + lo
                return gbt[g][0:m, e0:e0 + win]

            for g in range(ngrp):
                blks = list(range(gbounds[g], gbounds[g + 1]))
                glo = sblk[gbounds[g]]
                for i in blks:
                    m, win, pfirst = ms[i], wins[i], pfirsts[i]
                    o = sblk[i] - glo
                    off = pfirst - seg_lo[g]
                    pm = psump.tile([128, BPC, win], F32,
                                    name=f"pm{i}", tag=f"pm{i % 8}")
                    for c in range(NC2):
                        nc.tensor.matmul(
                            pm[0:m, :, :],
                            w_ap(g, c, o, m),
                            f_ap(g, c, off, win),
                            start=(c == 0),
                            stop=(c == NC2 - 1),
                            perf_mode=DR,
                        )
                    mloc = int(moffs[i] - moffs[gbounds[g]])
                    # DVE: fused mask-mult + accum straight out of PSUM
                    for bb in range(BPC):
                        sc = spool.tile([128, WINMAX], F32, tag=f"sv{bb}")
                        nc.vector.scalar_tensor_tensor(
                            sc[0:m, 0:win],
                            pm[0:m, bb, :],
                            0.0,
                            m_ap(g, m, mloc, win),
                            ADD,
                            MULT,
                            accum_out=zAll[0:m,
                                           BPC * i + bb:BPC * i + bb + 1],
                        )
            # z stores ride the otherwise-idle scalar queue
            zcuts = sorted(set((0, 16, 24, nblk - 2, nblk - 1, nblk)))
            zcuts = [c for c in zcuts if 0 <= c <= nblk]
            for a, bnd in zip(zcuts[:-1], zcuts[1:]):
                sl = slice(BPC * a, BPC * bnd)
                nc.scalar.dma_start(z_h[:, sl], zAll[:, sl])

    _split_waits(nc)

    # ---- run on 8 cores: core id = bg*2 + dh ----
    xq = (x.reshape(B, D // 128, 128, P) * sx).astype(F8_NP)
    in_maps = []
    for core in range(NCORES):
        bg, dh = core // NDH, core % NDH
        blkx = xq[BPC * bg:BPC * (bg + 1),
                  4 * dh:4 * (dh + 1)].reshape(BPC, NC2, 2, 128, P)
        im = {}
        for g in range(ngrp):
            lo, w_ = seg_lo[g], seg_w[g]
            seg = np.zeros((BPC, NC2, 2, 128, w_), F8_NP)
            hi = min(P, lo + w_)
            seg[:, :, :, :, :hi - lo] = blkx[:, :, :, :, lo:hi]
            fpart = np.ascontiguousarray(
                seg.transpose(3, 1, 2, 0, 4)).reshape(128, -1).view(np.uint8)
            wpart = np.ascontiguousarray(
                wgrps[dh][g]).reshape(128, -1).view(np.uint8)
            mlo, mhi = int(moffs[gbounds[g]]), int(moffs[gbounds[g + 1]])
            mpart = np.ascontiguousarray(mask_all[:, mlo:mhi]).view(np.uint8)
            buf = np.concatenate([wpart, fpart, mpart], axis=1)
            assert buf.shape[1] == gcols[g], (buf.shape, gcols[g])
            im[f"gq{g}"] = buf.view(F8_NP)
        in_maps.append(im)
    res = run_bass_kernel_spmd(nc, in_maps, core_ids=list(range(NCORES)))

    # ---- assemble: add D-halves, bias, elu(y)+1 ----
    y = np.empty((B, N), np.float32)
    for bg in range(NBG):
        z = res.results[NDH * bg]["z"] + res.results[NDH * bg + 1]["z"]
        for i, (s, e) in enumerate(blocks):
            idx = order[s:e]
            m = e - s
            y[BPC * bg:BPC * (bg + 1), idx] = z[0:m, BPC * i:BPC * (i + 1)].T
    y += b
    return np.where(y > 0, y + np.float32(1.0),
                    np.exp(np.minimum(y, np.float32(0.0)))).astype(np.float32)


# revision 9
# speedup vs baseline: 1.0666x; 1.0666x over previous
"""PoissonGaussianReadout forward on 8 trn2 NeuronCores.

Math (eval mode): each neuron n samples feat[b] (a [36,36,1024] image per
batch, 1024 = C*T channels) bilinearly at a fixed point mu[n], then takes a
per-neuron dot with W[n,:], adds b[n], applies elu(y)+1.

Strategy:
  - Hybrid shard 4x2: 8 cores = 4 batch-groups (4 batches each) x 2 halves
    of the contraction dim D (512 channels each).  Cores emit LINEAR
    partial sums; the host adds the halves, bias, and elu on [16,4096].
  - fp8(e4m3) x and W with DoubleRow matmuls; per-neuron dequant folded
    into a bf16 mask.
  - Sort neurons by bilinear base cell p00; 32 blocks of 128 neurons,
    each spanning a window of <=93 flat positions.  Two DoubleRow
    matmuls per block: psum[n, (b,j)] += Wblk^T @ feat-window.
  - The bilinear mask-reduce runs entirely on DVE as fused
    scalar_tensor_tensor+accum chains straight out of PSUM (~156ns per
    (block,batch)).  Measured on HW, every offload (GpSimd tensor_tensor
    mult ~775ns/blk, Act activation+accum ~380+279ns/batch, DVE
    tensor_reduce ~470ns/blk with op-mixing penalties) loses to this.
  - DMA: ONE need-ordered, ungated stream on the sync-engine HWDGE queue.
    Each block group ships as a single combined [W|F|M] host buffer,
    DMA'd as TWO big half-entries: a lone queue entry only sustains
    ~0.2 MB/us while two concurrent entries reach the ~0.4 MB/us per-core
    cap, and more than ~8 total entries stall on Tile's DMA-semaphore
    reuse.  Ragged groups (2,4,9,9,8 blocks) start the pipeline early.
    The two HWDGE queues share one bandwidth pool, so the scalar queue
    carries only the (late, tiny) z stores.
"""
import sys
sys.path.insert(0, "/opt/trn_rl_repo")

import numpy as np

from concourse import bass, mybir, tile
from concourse.bass_utils import run_bass_kernel_spmd
import bass_rust

# problem constants
B, C, T, HH, WW = 16, 64, 16, 36, 36
N, D = 4096, C * T             # 4096 neurons, 1024 input dim
P = HH * WW                    # 1296 flat positions
NCORES = 8
NBG = 4                        # batch groups
NDH = 2                        # D halves
BPC = B // NBG                 # batches per core = 4
DH = D // NDH                  # channels per core = 512
NC2 = DH // 256                # 2 double-subtile (256-chan) passes per core
PAD = 38                       # max corner offset (37) + 1
WINMAX = 128                   # psum bank: BPC*WIN <= 512 fp32
GSIZES = (2, 4, 9, 9, 8)       # ragged block groups: small first groups so
                               # the reduce pipeline starts earlier

F32 = mybir.dt.float32
BF16 = mybir.dt.bfloat16

import ml_dtypes
F8_DT = mybir.dt.float8e4
F8_NP = ml_dtypes.float8_e4m3   # max normal 240
F8_CAP = np.float32(224.0)


def _split_waits(nc, max_waits=1):
    """Walrus in this image allows only ONE sem wait per instruction.
    Hoist extra waits onto injected same-engine NoOps placed immediately
    before the owning instruction (same engine + program order => same
    semantics)."""
    k = 0
    for fn in nc.m.functions:
        for blk in fn.blocks:
            insts = blk.instructions
            out = []
            for inst in insts:
                si = inst.sync_info
                if si is not None and si.on_wait and len(si.on_wait) > max_waits:
                    waits = list(si.on_wait)
                    for w in waits[:-max_waits]:
                        nop = mybir.InstNoOp(name=f"I-wsplit-{k}", ins=[], outs=[])
                        k += 1
                        nop.engine = inst.engine
                        nop.sync_info = bass_rust.SyncInfo(
                            on_wait=[w], on_update=[]
                        )
                        out.append(nop)
                    si.on_wait = waits[-max_waits:]
                    inst.sync_info = si
                out.append(inst)
            if len(out) != len(insts):
                insts.clear()
                insts.extend(out)


def _bilinear_tables(mu):
    """Per-neuron base cell p00, corner offsets (4) in {0,1,36,37}, corner
    weights (4), replicating reference float32 arithmetic exactly."""
    one, half = np.float32(1.0), np.float32(0.5)
    g = np.clip(mu.astype(np.float32), -one, one)
    ix = (g[:, 0] + one) * np.float32(WW * 0.5) - half
    iy = (g[:, 1] + one) * np.float32(HH * 0.5) - half
    x0 = np.floor(ix)
    y0 = np.floor(iy)
    wx1 = ix - x0
    wx0 = one - wx1
    wy1 = iy - y0
    wy0 = one - wy1

    xs = [x0, x0 + one]
    ys = [y0, y0 + one]
    wxs = [wx0, wx1]
    wys = [wy0, wy1]

    x0c = np.clip(x0, 0, WW - 1).astype(np.int64)
    y0c = np.clip(y0, 0, HH - 1).astype(np.int64)
    p00 = y0c * WW + x0c

    offs = np.zeros((4, N), np.int64)
    wgts = np.zeros((4, N), np.float32)
    k = 0
    for a in range(2):          # y corner
        for bb in range(2):     # x corner
            xx, yy = xs[bb], ys[a]
            valid = (xx >= 0) & (xx <= WW - 1) & (yy >= 0) & (yy <= HH - 1)
            xi = np.clip(xx, 0, WW - 1).astype(np.int64)
            yi = np.clip(yy, 0, HH - 1).astype(np.int64)
            offs[k] = yi * WW + xi - p00
            wgts[k] = (wys[a] * wxs[bb]) * valid.astype(np.float32)
            k += 1
    assert offs.min() >= 0 and offs.max() <= 37
    return p00, offs, wgts


def _make_blocks(p00_sorted):
    """Greedy blocks of <=128 sorted neurons with window <= WINMAX."""
    blocks = []  # (start, end) into sorted order
    s = 0
    n = len(p00_sorted)
    while s < n:
        pfirst = p00_sorted[s]
        e = s
        while e < n and e - s < 128 and (p00_sorted[e] - pfirst) + PAD <= WINMAX:
            e += 1
        blocks.append((s, e))
        s = e
    return blocks


def kernel(x, mu, sigma, W, b):
    x = np.ascontiguousarray(x, dtype=np.float32)
    W = np.ascontiguousarray(W, dtype=np.float32)
    b = np.asarray(b, dtype=np.float32)

    p00, offs, wgts = _bilinear_tables(mu)
    order = np.argsort(p00, kind="stable")
    p00s = p00[order]
    blocks = _make_blocks(p00s)
    nblk = len(blocks)
    gbounds = [0]
    for gs in GSIZES:
        gbounds.append(min(gbounds[-1] + gs, nblk))
    while gbounds[-1] < nblk:
        gbounds.append(min(gbounds[-1] + GSIZES[-1], nblk))
    gbounds = sorted(set(gbounds))
    ngrp = len(gbounds) - 1

    # ---- fp8 quantization: global x scale, per-neuron W scale ----
    sx = F8_CAP / np.float32(max(np.abs(x).max(), 1e-30))
    sw = F8_CAP / np.maximum(np.abs(W).max(axis=1), 1e-30).astype(np.float32)
    Wq = (W * sw[:, None]).astype(F8_NP)    # [N, D]
    dequant = 1.0 / (sw * sx)               # [N] folded into the mask

    # per-block host data
    wins, pfirsts, ms, sblk = [], [], [], []
    mparts = []
    for i, (s, e) in enumerate(blocks):
        idx = order[s:e]
        m = e - s
        pfirst = int(p00s[s])
        win = int(p00s[e - 1]) - pfirst + PAD
        ms.append(m)
        pfirsts.append(pfirst)
        wins.append(win)
        sblk.append(s)
        # mask [128, win], fp8 dequant folded in
        mk = np.zeros((128, win), np.float32)
        rel = (p00[idx] - pfirst)  # [m]
        for k in range(4):
            np.add.at(mk[:m], (np.arange(m), rel + offs[k][idx]),
                      wgts[k][idx] * dequant[idx])
        mparts.append(mk)
    sblk.append(N)

    mask_all = np.ascontiguousarray(
        np.concatenate(mparts, axis=1)).astype(ml_dtypes.bfloat16)
    moffs = np.cumsum([0] + [w for w in wins])
    mtot = int(mask_all.shape[1])

    # feat segments: one per block group; window-union of its blocks
    seg_lo, seg_w = [], []
    for g in range(ngrp):
        lo = pfirsts[gbounds[g]]
        hi = max(pfirsts[i] + wins[i] for i in range(gbounds[g], gbounds[g + 1]))
        seg_lo.append(lo)
        seg_w.append(hi - lo)

    # W packed per group with one contiguous row per partition:
    # group layout [128, NC2, 2, sum_m(group)].
    Ws = Wq[order]                          # [N, D] sorted
    gms = [sblk[gbounds[g + 1]] - sblk[gbounds[g]] for g in range(ngrp)]
    wgrps = []                              # [dh][g] -> [128, NC2, 2, gm]
    for dh in range(NDH):
        wl = (Ws[:, dh * DH:(dh + 1) * DH].T        # [512, N]
              .reshape(NC2, 2, 128, N).transpose(2, 0, 1, 3))  # [128,NC2,2,N]
        parts = []
        for g in range(ngrp):
            lo, hi = sblk[gbounds[g]], sblk[gbounds[g + 1]]
            parts.append(np.ascontiguousarray(wl[:, :, :, lo:hi]))
        wgrps.append(parts)

    # per-group combined stream buffer: [ W | F | M ] bytes, one contiguous
    # row per partition, DMA'd as two big halves -> only 2 queue entries per
    # group (Tile has ~8 DMA sems; more entries serialize on sem reuse and
    # starve the queue).
    woffs, foffs, muoffs, gcols = [], [], [], []
    for g in range(ngrp):
        wb = NC2 * 2 * gms[g]
        fb = NC2 * 2 * BPC * seg_w[g]
        mb = 2 * int(moffs[gbounds[g + 1]] - moffs[gbounds[g]])
        woffs.append(0)
        foffs.append(wb)
        muoffs.append(wb + fb)
        gcols.append(wb + fb + mb)

    # ---- build the Bass program (same for all cores) ----
    nc = bass.Bass()
    gq_hs = [nc.declare_dram_parameter(f"gq{g}", [128, gcols[g]], F8_DT,
                                       isOutput=False)
             for g in range(ngrp)]
    z_h = nc.declare_dram_parameter("z", [128, BPC * nblk], F32, isOutput=True)

    ADD = mybir.AluOpType.add
    MULT = mybir.AluOpType.mult
    DR = mybir.MatmulPerfMode.DoubleRow

    with tile.TileContext(nc) as tc:
        with (
            tc.tile_pool(name="gq", bufs=1) as gqp,
            tc.tile_pool(name="spool", bufs=4) as spool,
            tc.tile_pool(name="zpool", bufs=1) as zpool,
            tc.tile_pool(name="psum", bufs=1, space="PSUM") as psump,
        ):
            gts = [gqp.tile([128, gcols[g]], F8_DT, name=f"gq{g}")
                   for g in range(ngrp)]
            gbt = [gt.bitcast(BF16) for gt in gts]   # bf16 view for masks
            zAll = zpool.tile([128, BPC * nblk], F32)

            # ONE need-ordered, ungated stream on the sync queue, cut into
            # ~0.45MB entries: a lone entry only sustains ~0.2 MB/us (need 2
            # concurrent for the 0.4 cap), while bigger entries take >1.2us
            # to trigger, which breaks the 2-deep pipeline at group
            # boundaries.  ~15 entries is fine: Tile's ~10 DMA sems recycle
            # long before their reuse.
            ECOLS = 3600  # ~0.45MB per entry
            for g in range(ngrp):
                nsl = max(1, (gcols[g] + ECOLS - 1) // ECOLS)
                cuts = [((gcols[g] * k) // nsl) & ~1 for k in range(nsl)]
                cuts.append(gcols[g])
                for a, bnd in zip(cuts[:-1], cuts[1:]):
                    nc.sync.dma_start(gts[g][:, a:bnd], gq_hs[g][:, a:bnd])

            def w_ap(g, c, o, m):
                # [128, 2, m] fp8 view of the W part of group g, chunk c
                base = gts[g][:, 0:1]
                pstr = list(base.ap)[0]
                gm = gms[g]
                return bass.AP(base.tensor,
                               base.offset + woffs[g] + c * 2 * gm + o,
                               [list(pstr), [gm, 2], [1, m]])

            def f_ap(g, c, off, win):
                # [128, 2, BPC, win] fp8 view of the feat part of group g
                base = gts[g][:, 0:1]
                pstr = list(base.ap)[0]
                sw_ = seg_w[g]
                return bass.AP(base.tensor,
                               base.offset + foffs[g]
                               + c * 2 * BPC * sw_ + off,
                               [list(pstr), [BPC * sw_, 2], [sw_, BPC],
                                [1, win]])

            def m_ap(g, m, lo, win):
                # [m, win] bf16 view of the mask part of group g
                e0 = muoffs[g] // 2 + lo
                return gbt[g][0:m, e0:e0 + win]

            for g in range(ngrp):
                blks = list(range(gbounds[g], gbounds[g + 1]))
                glo = sblk[gbounds[g]]
                for i in blks:
                    m, win, pfirst = ms[i], wins[i], pfirsts[i]
                    o = sblk[i] - glo
                    off = pfirst - seg_lo[g]
                    pm = psump.tile([128, BPC, win], F32,
                                    name=f"pm{i}", tag=f"pm{i % 8}")
                    for c in range(NC2):
                        nc.tensor.matmul(
                            pm[0:m, :, :],
                            w_ap(g, c, o, m),
                            f_ap(g, c, off, win),
                            start=(c == 0),
                            stop=(c == NC2 - 1),
                            perf_mode=DR,
                        )
                    mloc = int(moffs[i] - moffs[gbounds[g]])
                    # DVE: fused mask-mult + accum straight out of PSUM
                    for bb in range(BPC):
                        sc = spool.tile([128, WINMAX], F32, tag=f"sv{bb}")
                        nc.vector.scalar_tensor_tensor(
                            sc[0:m, 0:win],
                            pm[0:m, bb, :],
                            0.0,
                            m_ap(g, m, mloc, win),
                            ADD,
                            MULT,
                            accum_out=zAll[0:m,
                                           BPC * i + bb:BPC * i + bb + 1],
                        )
            # z stores on the sync queue (warm; idle once inputs are in --
            # the scalar queue would pay its ~3us cold-start right at the
            # end of the kernel)
            zcuts = sorted(set((0, 16, 24, nblk - 2, nblk - 1, nblk)))
            zcuts = [c for c in zcuts if 0 <= c <= nblk]
            for a, bnd in zip(zcuts[:-1], zcuts[1:]):
                sl = slice(BPC * a, BPC * bnd)
                nc.sync.dma_start(z_h[:, sl], zAll[:, sl])

    _split_waits(nc)

    # ---- run on 8 cores: core id = bg*2 + dh ----
    xq = (x.reshape(B, D // 128, 128, P) * sx).astype(F8_NP)
    in_maps = []
    for core in range(NCORES):
        bg, dh = core // NDH, core % NDH
        blkx = xq[BPC * bg:BPC * (bg + 1),
                  4 * dh:4 * (dh + 1)].reshape(BPC, NC2, 2, 128, P)
        im = {}
        for g in range(ngrp):
            lo, w_ = seg_lo[g], seg_w[g]
            seg = np.zeros((BPC, NC2, 2, 128, w_), F8_NP)
            hi = min(P, lo + w_)
            seg[:, :, :, :, :hi - lo] = blkx[:, :, :, :, lo:hi]
            fpart = np.ascontiguousarray(
                seg.transpose(3, 1, 2, 0, 4)).reshape(128, -1).view(np.uint8)
            wpart = np.ascontiguousarray(
                wgrps[dh][g]).reshape(128, -1).view(np.uint8)
            mlo, mhi = int(moffs[gbounds[g]]), int(moffs[gbounds[g + 1]])
            mpart = np.ascontiguousarray(mask_all[:, mlo:mhi]).view(np.uint8)
            buf = np.concatenate([wpart, fpart, mpart], axis=1)
            assert buf.shape[1] == gcols[g], (buf.shape, gcols[g])
            im[f"gq{g}"] = buf.view(F8_NP)
        in_maps.append(im)
    res = run_bass_kernel_spmd(nc, in_maps, core_ids=list(range(NCORES)))

    # ---- assemble: add D-halves, bias, elu(y)+1 ----
    y = np.empty((B, N), np.float32)
    for bg in range(NBG):
        z = res.results[NDH * bg]["z"] + res.results[NDH * bg + 1]["z"]
        for i, (s, e) in enumerate(blocks):
            idx = order[s:e]
            m = e - s
            y[BPC * bg:BPC * (bg + 1), idx] = z[0:m, BPC * i:BPC * (i + 1)].T
    y += b
    return np.where(y > 0, y + np.float32(1.0),
                    np.exp(np.minimum(y, np.float32(0.0)))).astype(np.float32)
